# revision 15
# baseline (speedup 1.0000x reference)
"""LFQ quantizer (vq_codebook) Trainium2 Bass kernel.

Problem: x [8, 1024, 14] f32.  Codebook = all 2^14 sign patterns (16384 x 14).
Outputs (matching the reference):
  q_out            [8, 1024, 14] f32   = where(x > 0, 1, -1)
  entropy_aux_loss scalar f32          = sample_entropy - avg_entropy
  sample_entropy   scalar f32          = mean_n H(softmax(200 * x_n . C^T))
  avg_entropy      scalar f32          = -sum_j q_j log(q_j + 1e-5),
                                         q = mean_n softmax(200 * x_n . C^T)
  commit_loss      scalar f32          = mean((x - q_out)^2) = mean((|x|-1)^2)

Math used by the kernel (softmax over ALL sign patterns factorizes):
  probs[n, j] = exp(200 * dot(x_n, c_j) + negbias_n)            (exactly)
  negbias_n   = -sum_d (200*|x_nd| + softplus(-400*|x_nd|))
  H_n         = sum_d [ softplus(-400|x_nd|) + 400|x_nd| * sigmoid(-400|x_nd|) ]

Sharding: data-parallel over batch b, one batch row (1024 samples) per core.
Each core computes its partial column-sums of probs ([16384]), partial
sample-entropy / commit sums, and its q_out slice.  Host does the final
(tiny) 8-way reductions and the 16384-bin entropy.

Logit precision: the matmul uses an fp16 hi/lo split of x stacked along the
contraction dim (K=28).  The codebook is exactly representable in fp16, so
the PSUM result is fp32-accurate while running at full bf16-class speed
(matmul time depends on N, not K<=128).
"""

import numpy as np

import concourse.bass as bass
import concourse.mybir as mybir
from concourse.bass_utils import run_bass_kernel_spmd
from concourse.tile import TileContext

B, L, D = 8, 1024, 14
K = 16384
N_CORES = 8
ST = 8            # sample tiles of 128 per core
NG = 8            # code groups per core pass
GW = K // NG      # group width = 2048 codes (4 PSUM banks per tile)
EPS = 1e-5
# Stationary layout: hi rows at partitions 0-13, lo rows at 32-45 (compute
# engines may only start at partition 0/32/64/96); rows 14-31 are zero on
# both the x side and the codebook side.
KSTA = 46
LO0 = 32

f32 = mybir.dt.float32
f16 = mybir.dt.float16
AF = mybir.ActivationFunctionType
OP = mybir.AluOpType
AX = mybir.AxisListType


def _build_codebook():
    codes = np.arange(K, dtype=np.int64)
    mask = 2 ** np.arange(D, dtype=np.int64)
    bits = ((codes[:, None] & mask[None, :]) != 0).astype(np.float32)
    cbT = (bits * 2.0 - 1.0).T  # [14, 16384]
    cb = np.zeros((KSTA, K), dtype=np.float16)
    cb[0:D] = cbT
    cb[LO0:LO0 + D] = cbT
    return cb


def _build_nc():
    # use_seq_codegen lowers multi-semaphore waits into sequencer
    # instructions; this walrus build's setupSyncWait only accepts one
    # sync-wait per engine instruction.
    nc = bass.Bass(trn_type="TRN2", use_seq_codegen=True)
    xp_d = nc.dram_tensor("xp", [128, ST * D], f32, kind="ExternalInput")
    xt_d = nc.dram_tensor("xt", [D, L], f32, kind="ExternalInput")
    cb_d = nc.dram_tensor("cb", [KSTA, K], f16, kind="ExternalInput")
    q_d = nc.dram_tensor("q", [128, ST * D], f32, kind="ExternalOutput")
    avgp_d = nc.dram_tensor("avgp", [1, K], f32, kind="ExternalOutput")
    ent_d = nc.dram_tensor("ent", [128, 1], f32, kind="ExternalOutput")
    cmt_d = nc.dram_tensor("cmt", [128, 1], f32, kind="ExternalOutput")

    with TileContext(nc) as tc:
        with (
            tc.tile_pool(name="const", bufs=1) as cpool,
            tc.tile_pool(name="work", bufs=1) as wpool,
            tc.tile_pool(name="probs", bufs=3) as ppool,
            tc.tile_pool(name="mm", bufs=2, space="PSUM") as mmpool,
        ):
            # ---- loads ----
            cb = cpool.tile([KSTA, K], f16, name="cb")
            for g in range(NG):
                nc.sync.dma_start(cb[:, g * GW:(g + 1) * GW],
                                  cb_d[:, g * GW:(g + 1) * GW])
            xt = cpool.tile([D, L], f32, name="xt")
            nc.sync.dma_start(xt, xt_d[:, :])
            xp = cpool.tile([128, ST * D], f32, name="xp")
            nc.sync.dma_start(xp, xp_d[:, :])

            # ---- fp16 hi/lo split of x^T, stacked on K: hl [46, 1024] ----
            hl = cpool.tile([KSTA, L], f16, name="hl")
            nc.vector.memset(hl, 0.0)
            nc.vector.tensor_copy(hl[0:D, :], xt)           # hi = f16(x)
            hi32 = wpool.tile([D, L], f32, name="hi32")
            nc.scalar.copy(hi32, hl[0:D, :])
            lo32 = wpool.tile([D, L], f32, name="lo32")
            nc.vector.tensor_tensor(lo32, xt, hi32, OP.subtract)
            nc.vector.tensor_copy(hl[LO0:LO0 + D, :], lo32)  # lo = f16(x - hi)

            # ---- per-sample stats on xp [128, 112] ----
            aa = wpool.tile([128, ST * D], f32, name="aa")
            nc.scalar.activation(aa, xp, AF.Abs)  # |x|
            e1 = wpool.tile([128, ST * D], f32, name="e1")
            nc.scalar.activation(e1, aa, AF.Exp, scale=-400.0)      # exp(-400|x|)
            lp = wpool.tile([128, ST * D], f32, name="lp")
            nc.scalar.activation(lp, e1, AF.Ln, bias=1.0)           # softplus(-400|x|)

            # negbias per sample: nb[p, st] = sum_d -(200|x| + lp)
            nbe = wpool.tile([128, ST * D], f32, name="nbe")
            nc.vector.tensor_scalar(nbe, aa, -200.0, None, OP.mult)
            nc.vector.tensor_tensor(nbe, nbe, lp, OP.subtract)
            nb = cpool.tile([128, ST], f32, name="nb")
            nc.vector.tensor_reduce(
                nb, nbe.rearrange("p (s d) -> p s d", d=D), axis=AX.X, op=OP.add)

            # per-dim entropy: H = lp + (400|x|) * sigmoid(-400|x|),
            # sigmoid(-400|x|) = e1 / (1 + e1)
            wv = wpool.tile([128, ST * D], f32, name="wv")
            nc.vector.tensor_scalar(wv, e1, 1.0, None, OP.add)
            rv = wpool.tile([128, ST * D], f32, name="rv")
            nc.vector.reciprocal(rv, wv)
            sg = wpool.tile([128, ST * D], f32, name="sg")
            nc.vector.tensor_tensor(sg, e1, rv, OP.mult)
            hh = wpool.tile([128, ST * D], f32, name="hh")
            nc.vector.tensor_scalar(hh, aa, 400.0, None, OP.mult)
            nc.vector.tensor_tensor(hh, hh, sg, OP.mult)
            nc.vector.tensor_tensor(hh, hh, lp, OP.add)
            entt = wpool.tile([128, 1], f32, name="entt")
            nc.vector.tensor_reduce(entt, hh, axis=AX.X, op=OP.add)
            nc.sync.dma_start(ent_d[:, :], entt)

            # commit partial: sum_d (|x| - 1)^2
            cv = wpool.tile([128, ST * D], f32, name="cv")
            nc.vector.tensor_scalar(cv, aa, -1.0, None, OP.add)
            nc.vector.tensor_tensor(cv, cv, cv, OP.mult)
            cmtt = wpool.tile([128, 1], f32, name="cmtt")
            nc.vector.tensor_reduce(cmtt, cv, axis=AX.X, op=OP.add)
            nc.sync.dma_start(cmt_d[:, :], cmtt)

            # q_out = sign(x)
            qt = wpool.tile([128, ST * D], f32, name="qt")
            nc.scalar.activation(qt, xp, AF.Sign)
            nc.sync.dma_start(q_d[:, :], qt)

            # ---- main loop: probs and their column-sum accumulation ----
            acc = cpool.tile([128, K], f16, name="acc")
            for st in range(ST):
                sta = hl[:, st * 128:(st + 1) * 128]  # [46, 128] stationary
                for g in range(NG):
                    pt = mmpool.tile([128, GW], f32, name="pt")
                    for j in range(GW // 512):
                        c0 = g * GW + j * 512
                        nc.tensor.matmul(
                            pt[:, j * 512:(j + 1) * 512],
                            lhsT=sta,
                            rhs=cb[:, c0:c0 + 512],
                            start=True, stop=True)
                    pr = ppool.tile([128, GW], f16, name="pr")
                    nc.scalar.activation(
                        pr, pt, AF.Exp, bias=nb[:, st:st + 1], scale=200.0)
                    asl = acc[:, g * GW:(g + 1) * GW]
                    if st == 0:
                        nc.vector.tensor_copy(asl, pr)
                    else:
                        nc.vector.tensor_tensor(asl, asl, pr, OP.add)

            # ---- tail: column sums over the 128 partitions via ones-matmul ----
            onesw = cpool.tile([128, 1], f16, name="onesw")
            nc.vector.memset(onesw, 1.0)
            avgsb = cpool.tile([1, K], f32, name="avgsb")
            for r in range(NG):
                pa = mmpool.tile([128, GW], f32, name="pt")  # reuse mm slots
                for j in range(GW // 512):
                    t0 = r * GW + j * 512
                    nc.tensor.matmul(
                        pa[0:1, j * 512:(j + 1) * 512],
                        lhsT=onesw,
                        rhs=acc[:, t0:t0 + 512],
                        start=True, stop=True)
                # split the PSUM->SBUF evacuations across ACT and DVE so the
                # serialized tail is halved
                if r % 2 == 0:
                    nc.scalar.copy(avgsb[:, r * GW:(r + 1) * GW], pa[0:1, :])
                else:
                    nc.vector.tensor_copy(avgsb[:, r * GW:(r + 1) * GW],
                                          pa[0:1, :])
            nc.sync.dma_start(avgp_d[:, :], avgsb)

    return nc


def _split_multiwaits(bir):
    """This walrus build's setupSyncWait accepts only ONE sync-wait per
    engine instruction; Tile emits several.  Split the extras onto injected
    same-engine EventSemaphore ops directly before the instruction (same
    stream position => identical semantics)."""
    for f in bir["functions"]:
        for b in f["blocks"]:
            new = []
            for inst in b["instructions"]:
                si = inst.get("sync_info")
                if si and len(si.get("on_wait") or []) > 1:
                    waits = si["on_wait"]
                    for k, w in enumerate(waits[:-1]):
                        new.append({
                            "debug": inst.get("debug", 0),
                            "engine": inst["engine"],
                            "ins": [], "outs": [],
                            "name": inst["name"] + f"_w{k}",
                            "opcode": "EventSemaphore",
                            "sync_info": {"on_update": [], "on_wait": [w]},
                        })
                    si["on_wait"] = [waits[-1]]
                new.append(inst)
            b["instructions"] = new
    return bir


_CACHED = {}


def _get_nc():
    if "nc" not in _CACHED:
        import orjson
        nc = _build_nc()
        orig_to_json = nc.to_json_bytes

        def patched_to_json():
            return orjson.dumps(_split_multiwaits(orjson.loads(orig_to_json())))

        nc.to_json_bytes = patched_to_json
        _CACHED["nc"] = nc
    return _CACHED["nc"]


def _make_in_maps(x):
    cb46 = _build_codebook()
    in_maps = []
    for c in range(N_CORES):
        xc = np.ascontiguousarray(x[c], dtype=np.float32)       # [1024, 14]
        xp = np.ascontiguousarray(
            xc.reshape(ST, 128, D).transpose(1, 0, 2).reshape(128, ST * D))
        xt = np.ascontiguousarray(xc.T)                          # [14, 1024]
        in_maps.append({"xp": xp, "xt": xt, "cb": cb46})
    return in_maps


def _finalize(x, results):
    q_out = np.empty((B, L, D), dtype=np.float32)
    avg_sum = np.zeros(K, dtype=np.float64)
    ent_sum = 0.0
    cmt_sum = 0.0
    for c in range(N_CORES):
        r = results[c]
        q_out[c] = (r["q"].reshape(128, ST, D).transpose(1, 0, 2)
                    .reshape(L, D))
        avg_sum += r["avgp"].reshape(K).astype(np.float64)
        ent_sum += float(r["ent"].sum())
        cmt_sum += float(r["cmt"].sum())

    n_samples = B * L
    avg_probs = avg_sum / n_samples
    avg_entropy = float(-np.sum(avg_probs * np.log(avg_probs + EPS)))
    sample_entropy = ent_sum / n_samples
    commit_loss = cmt_sum / (n_samples * D)
    entropy_aux_loss = sample_entropy - avg_entropy

    return (
        q_out,
        np.float32(entropy_aux_loss),
        np.float32(sample_entropy),
        np.float32(avg_entropy),
        np.float32(commit_loss),
    )


def _make_exec():
    """Build the sharded PJRT executable once (mirrors
    bass2jax.run_bass_via_pjrt, but reusable for repeated timed calls)."""
    if "exec" in _CACHED:
        return _CACHED["exec"]
    import jax
    from jax.sharding import Mesh, PartitionSpec
    from jax.experimental.shard_map import shard_map
    from concourse.bass2jax import (
        _bass_exec_p, install_neuronx_cc_hook, partition_id_tensor)

    nc = _get_nc()
    install_neuronx_cc_hook()
    partition_name = (nc.partition_id_tensor.name
                      if nc.partition_id_tensor else None)

    in_names, out_names, out_avals, zero_outs = [], [], [], []
    for alloc in nc.m.functions[0].allocations:
        if not isinstance(alloc, mybir.MemoryLocationSet):
            continue
        name = alloc.memorylocations[0].name
        if alloc.kind == "ExternalInput":
            if name != partition_name:
                in_names.append(name)
        elif alloc.kind == "ExternalOutput":
            shape = tuple(alloc.tensor_shape)
            dtype = mybir.dt.np(alloc.dtype)
            out_names.append(name)
            out_avals.append(jax.core.ShapedArray(shape, dtype))
            zero_outs.append(np.zeros(shape, dtype))
    n_params = len(in_names)
    n_outs = len(out_avals)
    all_in_names = list(in_names) + list(out_names)
    if partition_name is not None:
        all_in_names.append(partition_name)
    donate = tuple(range(n_params, n_params + n_outs))

    def _body(*args):
        operands = list(args)
        if partition_name is not None:
            operands.append(partition_id_tensor())
        outs = _bass_exec_p.bind(
            *operands,
            out_avals=tuple(out_avals),
            in_names=tuple(all_in_names),
            out_names=tuple(out_names),
            lowering_input_output_aliases=(),
            sim_require_finite=True,
            sim_require_nnan=True,
            nc=nc,
        )
        return tuple(outs)

    devices = jax.devices()[:N_CORES]
    mesh = Mesh(np.asarray(devices), ("core",))
    in_specs = (PartitionSpec("core"),) * (n_params + n_outs)
    out_specs = (PartitionSpec("core"),) * len(out_names)
    sharded = jax.jit(
        shard_map(_body, mesh=mesh, in_specs=in_specs, out_specs=out_specs,
                  check_rep=False),
        donate_argnums=donate, keep_unused=True)

    _CACHED["exec"] = (sharded, in_names, out_names, out_avals, zero_outs)
    return _CACHED["exec"]


def _run_device(x):
    """Execute on 8 cores; returns per-core result dicts."""
    import jax
    sharded, in_names, out_names, out_avals, zero_outs = _make_exec()
    in_maps = _make_in_maps(np.asarray(x, dtype=np.float32))
    concat_in = [
        np.concatenate([np.asarray(in_maps[c][n]) for c in range(N_CORES)],
                       axis=0)
        for n in in_names
    ]
    concat_zeros = [
        np.zeros((N_CORES * z.shape[0], *z.shape[1:]), z.dtype)
        for z in zero_outs
    ]
    out_arrs = sharded(*concat_in, *concat_zeros)
    return [
        {name: np.asarray(out_arrs[i]).reshape(N_CORES, *out_avals[i].shape)[c]
         for i, name in enumerate(out_names)}
        for c in range(N_CORES)
    ]


def bench(x, iters=20):
    """Return list of per-iteration wall times (s) for the device execution."""
    import time
    import jax
    sharded, in_names, out_names, out_avals, zero_outs = _make_exec()
    in_maps = _make_in_maps(np.asarray(x, dtype=np.float32))
    concat_in = [
        np.concatenate([np.asarray(in_maps[c][n]) for c in range(N_CORES)],
                       axis=0)
        for n in in_names
    ]
    dev_in = [jax.device_put(a) for a in concat_in]  # keep inputs resident
    times = []
    for _ in range(iters + 2):
        concat_zeros = [
            np.zeros((N_CORES * z.shape[0], *z.shape[1:]), z.dtype)
            for z in zero_outs
        ]
        t0 = time.perf_counter()
        out = sharded(*dev_in, *concat_zeros)
        jax.block_until_ready(out)
        times.append(time.perf_counter() - t0)
    return times[2:]  # drop warmup


def run_raw(x, trace=False):
    """Run the device kernel; returns (outputs_tuple, None)."""
    results = _run_device(x)
    return _finalize(x, results), None


def kernel(x):
    out, _ = run_raw(x)
    return out
